# revision 29
# baseline (speedup 1.0000x reference)
"""DSAM (dual spatial/channel attention) Bass kernel for Trainium2, 8 cores.

Sharding: core c handles batch b=c//4, query-row quarter qi=c%4
(1024 of the 4096 spatial positions). Spatial attention is fused
flash-style (scores -> exp -> weighted sum of V, normalization folded in
via an appended ones-row of V), so the [HW,HW] affinity never touches HBM.
The channel branch (full-image 3x3 conv + 64x64 gram) is computed
redundantly per core.

All inputs are packed into one [65, NIN] array so the whole load is a
single DMA (one semaphore) - PE weight-load instructions only tolerate a
single sync wait, so every matmul operand must trace to one producer
semaphore.

Hardcoded shapes: B=2, C=64, H=W=64, Cq=8.
"""

import numpy as np

EPS = 1e-5
B, C, H, W = 2, 64, 64, 64
HW = H * W
Cq = C // 8
NPAD = 66 * 66 + 2        # per-channel padded flat length (+2 sentinels)
XS_LEN = 18 * 66 + 2      # 18 padded rows slab (+2 sentinels)
NQ = 1024                 # query positions per core

# offsets into the packed input
O_XPA = 0
O_XS = O_XPA + NPAD
O_WS = O_XS + XS_LEN
O_WC = O_WS + 576
O_WQ = O_WC + 576
O_WK = O_WQ + Cq
O_WV = O_WK + Cq
O_WO = O_WV + 65
O_ID = O_WO + 64
O_OB = O_ID + 64
O_CG = O_OB + 1
NIN = O_CG + 1

_CACHE = {}


def _build():
    import concourse.bass as bass
    import concourse.tile as tile
    from concourse import mybir
    from contextlib import ExitStack

    fp = mybir.dt.float32
    f16 = mybir.dt.float16
    AX = mybir.AxisListType.X
    ALU = mybir.AluOpType
    ACTF = mybir.ActivationFunctionType

    nc = bass.Bass()
    in_d = nc.dram_tensor("allin", [65, NIN], fp, kind="ExternalInput")
    out_d = nc.dram_tensor("out", [64, NQ], fp, kind="ExternalOutput")

    with tile.TileContext(nc) as tc, ExitStack() as ctx:
        const = ctx.enter_context(tc.tile_pool(name="const", bufs=1))
        big = ctx.enter_context(tc.tile_pool(name="big", bufs=1))
        work = ctx.enter_context(tc.tile_pool(name="work", bufs=3))
        ps_s = ctx.enter_context(tc.tile_pool(name="ps_s", bufs=2, space="PSUM"))
        ps_u = ctx.enter_context(tc.tile_pool(name="ps_u", bufs=2, space="PSUM"))
        ps_m = ctx.enter_context(tc.tile_pool(name="ps_m", bufs=2, space="PSUM"))

        def fenced(pool, shape, tag):
            # record-keeping no-op wrapper; the wait-strip post-pass below
            # handles PSUM-slot-reuse wait overflow
            return pool.tile(shape, fp, tag=tag, name=tag), []

        allin = big.tile([65, NIN], fp)
        nc.gpsimd.dma_start(allin, in_d[:, :])

        xpa = allin[:, O_XPA:O_XPA + NPAD]
        xs = allin[:, O_XS:O_XS + XS_LEN]
        ws = allin[:, O_WS:O_WS + 576].rearrange("c (t o) -> c t o", t=9)
        wc = allin[:, O_WC:O_WC + 576].rearrange("c (t o) -> c t o", t=9)

        # valid-position views: flat index of pixel (r, w) is (r+1)*66+(w+1)+1
        xpa_v = xpa[:, 68:68 + 64 * 66].rearrange("c (r w) -> c r w", w=66)[:, :, :64]
        xs_v = xs[:, 68:68 + 16 * 66].rearrange("c (r w) -> c r w", w=66)[:, :, :64]

        # DVE-produced copies: dense valid pixels + small weights, so matmuls
        # whose other operand is DVE-produced see a single semaphore.
        xdense = big.tile([65, HW], f16)
        nc.scalar.copy(xdense.rearrange("c (r w) -> c r w", w=64), xpa_v)
        xsdense = big.tile([65, NQ], f16)
        nc.scalar.copy(xsdense.rearrange("c (r w) -> c r w", w=64), xs_v)
        wq = const.tile([65, Cq], f16)
        nc.scalar.copy(wq, allin[:, O_WQ:O_WQ + Cq])
        wk = const.tile([65, Cq], f16)
        nc.scalar.copy(wk, allin[:, O_WK:O_WK + Cq])
        wv = const.tile([65, 65], f16)
        nc.scalar.copy(wv, allin[:, O_WV:O_WV + 65])
        wo = const.tile([64, 64], fp)
        nc.vector.tensor_copy(wo, allin[:64, O_WO:O_WO + 64])
        ident = const.tile([64, 64], fp)
        nc.vector.tensor_copy(ident, allin[:64, O_ID:O_ID + 64])
        ob = const.tile([64, 1], fp)
        nc.vector.tensor_copy(ob, allin[:64, O_OB:O_OB + 1])
        cg = const.tile([64, 1], fp)
        nc.vector.tensor_copy(cg, allin[:64, O_CG:O_CG + 1])

        xpa16 = big.tile([65, NPAD], f16)
        nc.vector.tensor_copy(xpa16, xpa)
        xs16 = big.tile([65, XS_LEN], f16)
        nc.vector.tensor_copy(xs16, xs)
        ws16 = const.tile([65, 9, 64], f16)
        nc.vector.tensor_copy(ws16, ws)
        wc16 = const.tile([65, 9, 64], f16)
        nc.vector.tensor_copy(wc16, wc)

        ones16 = const.tile([1, 64], f16)
        nc.vector.tensor_copy(ones16, xpa_v[64:65, 0, :])

        # persistent SBUF tensors
        k_sb = big.tile([Cq, HW], f16)
        q_sb = big.tile([Cq, NQ], f16)
        vT = big.tile([128, 32, 65], f16)
        cxf = big.tile([64, HW], fp)       # full-image channel-conv fmap (dense)
        fT = big.tile([128, 32, 64], f16)   # fmap transposed chunks
        sxq = big.tile([64, NQ], fp)       # spatial-conv output, our rows
        cxq = big.tile([64, NQ], fp)       # channel-conv output, our rows
        fuse = big.tile([64, NQ], fp)
        out_sb = big.tile([64, NQ + 4], fp)

        # ---------- k / q (1x1 convs; ACT evictions so S_T sees one sem) ----
        for blk in range(8):
            ps, rec = fenced(ps_s, [128, 1024], "S")
            rec.append((nc.tensor.matmul(ps[:Cq, :512], wk,
                        xdense[:, blk * 512:(blk + 1) * 512],
                        start=True, stop=True), 'PE'))
            rec.append((nc.scalar.copy(k_sb[:, blk * 512:(blk + 1) * 512],
                                       ps[:Cq, :512]), 'ACT'))
        for blk in range(2):
            ps, rec = fenced(ps_s, [128, 1024], "S")
            rec.append((nc.tensor.matmul(ps[:Cq, :512], wq,
                        xsdense[:, blk * 512:(blk + 1) * 512],
                        start=True, stop=True), 'PE'))
            rec.append((nc.scalar.copy(q_sb[:, blk * 512:(blk + 1) * 512],
                                       ps[:Cq, :512]), 'ACT'))

        # ---------- vT[j, c] = (x . Wv)[j, c], col 64 = ones ----------
        for grp in range(5):
            n_t = min(7, 32 - grp * 7)
            ps, rec = fenced(ps_s, [128, 1024], "S")
            for t in range(n_t):
                jo = grp * 7 + t
                rec.append((nc.tensor.matmul(ps[:, t * 65:(t + 1) * 65],
                            xdense[:, jo * 128:(jo + 1) * 128], wv,
                            start=True, stop=True), 'PE'))
            rec.append((nc.scalar.copy(vT[:, grp * 7:grp * 7 + n_t, :],
                                       ps[:, :n_t * 65]), 'ACT'))

        # ---------- full-image channel conv (padded-space accumulation) -----
        rows_done = 0
        while rows_done < 64:
            rows = min(7, 64 - rows_done)
            fsz = rows * 66
            p0 = rows_done * 66
            ps, rec = fenced(ps_m, [128, 512], "m")
            for tap in range(9):
                dy, dx = tap // 3, tap % 3
                off = dy * 66 + dx
                rec.append((nc.tensor.matmul(ps[:64, :fsz], wc16[:, tap, :],
                            xpa16[:, off + p0: off + p0 + fsz],
                            start=(tap == 0), stop=(tap == 8)), 'PE'))
            pv = ps[:64, :fsz].rearrange("c (r w) -> c r w", w=66)[:, :, 1:65]
            rec.append((nc.vector.tensor_scalar_max(
                cxf[:, rows_done * 64:(rows_done + rows) * 64], pv, 0.0), 'DVE'))
            rows_done += rows

        # ---------- fmap transpose chunks ----------
        for grp in range(4):
            ps, rec = fenced(ps_m, [128, 512], "m")
            for t in range(8):
                jo = grp * 8 + t
                rec.append((nc.tensor.transpose(ps[:, t * 64:(t + 1) * 64],
                            cxf[:, jo * 128:(jo + 1) * 128], ident), 'PE'))
            rec.append((nc.vector.tensor_copy(fT[:, grp * 8:(grp + 1) * 8, :],
                                              ps), 'DVE'))

        # ---------- our-rows convs (spatial WS -> sxq, channel WC -> cxq) ---
        for wmat, dst in ((ws16, sxq), (wc16, cxq)):
            for bi, rows in enumerate((7, 7, 2)):
                fsz = rows * 66
                p0 = (0, 462, 924)[bi]
                ps, rec = fenced(ps_m, [128, 512], "m")
                for tap in range(9):
                    dy, dx = tap // 3, tap % 3
                    off = dy * 66 + dx
                    rec.append((nc.tensor.matmul(ps[:64, :fsz], wmat[:, tap, :],
                                xs16[:, off + p0: off + p0 + fsz],
                                start=(tap == 0), stop=(tap == 8)), 'PE'))
                pv = ps[:64, :fsz].rearrange("c (r w) -> c r w", w=66)[:, :, 1:65]
                rec.append((nc.vector.tensor_scalar_max(
                    dst[:, p0 // 66 * 64:(p0 // 66 + rows) * 64], pv, 0.0), 'DVE'))

        # ---------- spatial attention (flash-style) ----------
        for ib in range(2):
            psU, recU = fenced(ps_u, [65, 512], "U")
            for rnd in range(16):
                psS, rec = fenced(ps_s, [128, 1024], "S")
                for hh in range(2):
                    jo = rnd * 2 + hh
                    rec.append((nc.tensor.matmul(
                        psS[:, hh * 512:(hh + 1) * 512],
                        k_sb[:, jo * 128:(jo + 1) * 128],
                        q_sb[:, ib * 512:(ib + 1) * 512],
                        start=True, stop=True), 'PE'))
                Et = work.tile([128, 1024], f16, tag="E")
                et_last = Et
                rec.append((nc.scalar.activation(Et, psS, ACTF.Exp), 'ACT'))
                for hh in range(2):
                    jo = rnd * 2 + hh
                    recU.append((nc.tensor.matmul(psU, vT[:, jo, :],
                                 Et[:, hh * 512:(hh + 1) * 512],
                                 start=(jo == 0), stop=(jo == 31)), 'PE'))
            rcp = work.tile([1, 512], f16, tag="rec")
            with nc.allow_low_precision(reason="1/denom broadcast via f16 matmul"):
                nc.vector.reciprocal(rcp, psU[64:65, :])
            U_sb = work.tile([64, 512], fp, tag="U_sb")
            nc.vector.tensor_copy(U_sb, psU[:64, :])
            # broadcast 1/denom across partitions via a K=1 matmul with ones
            psB, recB = fenced(ps_m, [128, 512], "m")
            recB.append((nc.tensor.matmul(psB[:64, :], ones16, rcp,
                                          start=True, stop=True), 'PE'))
            rec64 = work.tile([64, 512], fp, tag="rec64")
            recB.append((nc.vector.tensor_copy(rec64, psB[:64, :]), 'DVE'))
            fb = fuse[:, ib * 512:(ib + 1) * 512]
            nc.vector.tensor_mul(fb, U_sb, rec64)
            nc.vector.tensor_add(fb, fb, sxq[:, ib * 512:(ib + 1) * 512])

        # ---------- channel attention ----------
        psA_t, recA = fenced(ps_m, [128, 512], "m")
        psA = psA_t[:64, :64]
        for jo in range(32):
            recA.append((nc.tensor.matmul(psA, fT[:, jo, :], fT[:, jo, :],
                         start=(jo == 0), stop=(jo == 31)), 'PE'))
        Ac = work.tile([64, 64], fp, tag="ac_sb")
        recA.append((nc.vector.tensor_copy(Ac, psA), 'DVE'))
        mn = work.tile([64, 1], fp, tag="mn")
        nc.vector.tensor_reduce(mn, Ac, AX, ALU.min)
        Ec = work.tile([64, 64], fp, tag="ec")
        # exp(mn - Ac): softmax(max-Ac) == softmax(-Ac), stabilized by row min
        nc.scalar.activation(Ec, Ac, ACTF.Exp, bias=mn, scale=-1.0)
        sm = work.tile([64, 1], fp, tag="sm")
        nc.vector.reduce_sum(sm, Ec, AX)
        rc = work.tile([64, 1], fp, tag="rc")
        nc.vector.reciprocal(rc, sm)
        # Ec := Ec * (1/sum) * c_gamma
        nc.vector.tensor_scalar(Ec, Ec, rc, cg, ALU.mult, ALU.mult)
        psT_t, recT = fenced(ps_m, [128, 512], "m")
        psT = psT_t[:64, :64]
        recT.append((nc.tensor.transpose(psT, Ec, ident), 'PE'))
        ScT = work.tile([64, 64], fp, tag="sct_sb")
        recT.append((nc.vector.tensor_copy(ScT, psT), 'DVE'))
        for ib in range(2):
            psC_t, recC = fenced(ps_m, [128, 512], "m")
            psC = psC_t[:64, :]
            recC.append((nc.tensor.matmul(psC, ScT,
                         cxq[:, ib * 512:(ib + 1) * 512],
                         start=True, stop=True), 'PE'))
            fb = fuse[:, ib * 512:(ib + 1) * 512]
            recC.append((nc.vector.tensor_add(fb, fb, psC), 'DVE'))
            nc.vector.tensor_add(fb, fb, cxq[:, ib * 512:(ib + 1) * 512])

        # ---------- output 1x1 conv ----------
        for ib in range(2):
            psO_t, recO = fenced(ps_m, [128, 512], "m")
            psO = psO_t[:64, :]
            recO.append((nc.tensor.matmul(psO, wo,
                         fuse[:, ib * 512:(ib + 1) * 512],
                         start=True, stop=True), 'PE'))
            recO.append((nc.vector.tensor_scalar_add(
                out_sb[:, ib * 512:(ib + 1) * 512], psO, ob), 'DVE'))
        # funnel ACT's tail into out_sb so the output DMA transitively
        # covers every engine; the final drain then only waits on the DMA
        nc.vector.tensor_copy(out_sb[0:1, NQ:NQ + 4], et_last[0:1, 0:4])
        nc.gpsimd.dma_start(out_d[:, :], out_sb[:, :NQ])

    # Engine instructions encode at most one sync wait. Where Tile emitted
    # two, one is always the own-engine wait for a slot WAW/WAR; engine
    # queues are FIFO and the kept cross-engine wait transitively covers the
    # own-engine one (the slot's reader waited on those writers). Strip it.
    eng_sem = {'PE': 'PE', 'Activation': 'Activation', 'DVE': 'DVE',
               'Pool': 'Pool', 'SP': 'SP'}
    for blk in nc.m.functions[0].blocks:
        for ins in blk.instructions:
            si = ins.sync_info
            if si is None or len(si.on_wait) <= 1:
                continue
            eng = str(getattr(ins, 'engine', '')).replace('EngineType.', '')
            if eng not in ('PE', 'Activation', 'DVE', 'Pool'):
                continue  # SP/sync instructions allow many waits
            own = eng_sem.get(eng, eng)
            keep = [w for w in si.on_wait if not w.ant_name.startswith(own)]
            assert len(keep) == 1, \
                (ins.name, eng, [w.ant_name for w in si.on_wait])
            si.on_wait = keep
            ins.sync_info = si

    # Tail drains join every engine, but their wait budget is 1. The output
    # DMA transitively covers every engine (it reads out_sb, whose writers
    # funnel ACT/PE/DVE), so the drain only needs the out-DMA queue's sem.
    last_dma_sem = None
    for blk in nc.m.functions[0].blocks:
        for ins in blk.instructions:
            si = ins.sync_info
            if si is None:
                continue
            for u in si.on_update:
                if u.ant_name.startswith('DMA'):
                    last_dma_sem = u.ant_name
    for blk in nc.m.functions[0].blocks:
        for ins in blk.instructions:
            si = ins.sync_info
            if si is None or type(ins).__name__ != 'InstDrain':
                continue
            if len(si.on_wait) > 1:
                keep = [w for w in si.on_wait if w.ant_name == last_dma_sem]
                if keep:
                    si.on_wait = keep
                    ins.sync_info = si

    return nc


def _prep_host(inputs):
    x = np.asarray(inputs['x'], np.float32)

    def fold(Wc, bc, g, b_, m, v):
        sc = np.asarray(g) / np.sqrt(np.asarray(v) + EPS)
        return (np.asarray(Wc) * sc[:, None, None, None],
                (np.asarray(bc) - np.asarray(m)) * sc + np.asarray(b_))

    sWf, sbf = fold(inputs['sW'], inputs['sb'], inputs['s_g'], inputs['s_b'],
                    inputs['s_m'], inputs['s_v'])
    cWf, cbf = fold(inputs['cW'], inputs['cb'], inputs['c_g'], inputs['c_b'],
                    inputs['c_m'], inputs['c_v'])

    def taps(Wf, bf_):
        out = np.zeros((65, 9, 64), np.float32)
        for dy in range(3):
            for dx in range(3):
                out[:64, dy * 3 + dx, :] = Wf[:, :, dy, dx].T
        out[64, 4, :] = bf_
        return out.reshape(65, 9 * 64)

    base = np.zeros((65, NIN), np.float32)
    base[:, O_WS:O_WS + 576] = taps(sWf, sbf)
    base[:, O_WC:O_WC + 576] = taps(cWf, cbf)
    base[:64, O_WQ:O_WQ + Cq] = np.asarray(inputs['qW'])[:, :, 0, 0].T
    base[64, O_WQ:O_WQ + Cq] = np.asarray(inputs['qb'])
    base[:64, O_WK:O_WK + Cq] = np.asarray(inputs['kW'])[:, :, 0, 0].T
    base[64, O_WK:O_WK + Cq] = np.asarray(inputs['kb'])
    sg = float(np.asarray(inputs['s_gamma'])[0])
    base[:64, O_WV:O_WV + 64] = np.asarray(inputs['vW'])[:, :, 0, 0].T * sg
    base[64, O_WV:O_WV + 64] = np.asarray(inputs['vb']) * sg
    base[64, O_WV + 64] = 1.0
    base[:64, O_WO:O_WO + 64] = np.asarray(inputs['oW'])[:, :, 0, 0].T
    base[:64, O_ID:O_ID + 64] = np.eye(64, dtype=np.float32)
    base[:64, O_OB] = np.asarray(inputs['ob'])
    base[:64, O_CG] = float(np.asarray(inputs['c_gamma'])[0])

    in_maps = []
    for c in range(8):
        b, qi = c // 4, c % 4
        m = base.copy()
        xp = np.zeros((65, 66, 66), np.float32)
        xp[:64, 1:65, 1:65] = x[b]
        xp[64, 1:65, 1:65] = 1.0
        m[:, O_XPA + 1:O_XPA + 1 + 66 * 66] = xp.reshape(65, 66 * 66)
        m[:, O_XS + 1:O_XS + 1 + 18 * 66] = \
            xp[:, qi * 16:qi * 16 + 18, :].reshape(65, 18 * 66)
        in_maps.append({'allin': np.ascontiguousarray(m)})
    return in_maps


def kernel(**inputs):
    from concourse.bass_utils import run_bass_kernel_spmd
    if 'nc' not in _CACHE:
        _CACHE['nc'] = _build()
    nc = _CACHE['nc']
    in_maps = _prep_host(inputs)
    res = run_bass_kernel_spmd(nc, in_maps, core_ids=list(range(8)))
    out = np.zeros((B, C, H, W), np.float32)
    for c in range(8):
        b, qi = c // 4, c % 4
        out[b, :, qi * 16:(qi + 1) * 16, :] = \
            res.results[c]['out'].reshape(64, 16, 64)
    return out
